# revision 26
# baseline (speedup 1.0000x reference)
"""Multi-head dot-product attention (with per-head LayerNorm on q/k/v) on 8
Trainium2 NeuronCores.

Model: x[4, 2048, 1024], 16 heads x 64 dim, LN (no affine) applied per head to
q/k/v projections, softmax attention, output projection.

Sharding: core = (batch, query-half). Each core owns one batch and 1024 query
tokens; it computes k/v for the full 2048 keys of its batch (25% duplicated
work, zero collectives). Attention is invariant to key order, so the host
rotates tokens per core to make the program pure SPMD.

Schedule: the Scalar engine's exp stream (~265us of Activation busy time) is
the critical chain; everything is organized to start it early (~25us) and
never starve it:
 - DMA priority: xh, wq(hi+lo), xl, wk, wv - the cost model serializes all
   transfers on a shared DMA-engine device, so arrival order is what matters.
   Big loads go through the Pool SWDGE queue (cheap descriptor-gen, keeps SP
   free for the XBAR transposes); x goes through the ACT queue (idle then).
 - k and v drop the Wlo weight half (y = Whi@(xhi+xlo)): the ~0.4% weight
   quantization noise is invisible next to softmax smoothing, saves 28us of
   PE and an extra 16.6KB weight slot. q keeps all three fp8 terms.
 - prologue: q0-3, v0, v1, k0 project up front; first exp at ~25us.
 - frontier: qb0 attention kt-outer in growing steps (kt0, kt1 singletons,
   then a pair, then quads - pv chains accumulate in PSUM across a step, SBUF
   acc across steps). Every sub-unit (j,kt) emits scores -> one ~1.3us
   projection PIECE (half a k/v/q tile) -> pv, so the PE does projection work
   exactly while ACT runs that sub-unit's exp: no PE stall on the exp
   latency, no ACT starvation, and the remaining 29 projection tiles finish
   by mid-frontier.
 - query-block 1 then runs kt-inner sweeps with PSUM accumulation, qb0's
   out-projection tiles interleaved between sweeps to fill PE gaps. A
   fraction of qb1's exp units (5 of 16 kt per j) are offloaded from ACT to
   DVE as a bf16-bits Schraudolph exp (one tensor_scalar: y=a*s+b converted
   to int16 IS the bf16 pattern of exp(s/8); ~2% rel err on those probs,
   washed out by softmax normalization) - DVE is nearly idle in that phase
   while ACT is the bottleneck.
 - xT and the q/k/v weight pools close after the frontier; aT, wo and the
   out staging pool reuse their SBUF space.
 - projections run as fp8e4 DoubleRow matmuls (0.5 cycles/moving-row, two
   k-tiles per instruction), x/W host-split into e4m3 hi+lo halves and
   host-prescaled by (8, 256) to clear e4m3's subnormal floor - LN is
   scale-invariant, only eps follows.
 - pv is flipped: probs [128 keys, 128 tokens] is the STATIONARY operand and
   v [128 keys, 65] the MOVING one (PE cost ~ moving rows: 65 vs 512), with
   the softmax denominator as a 65th v column; output lands [token, d] so
   1/l is a per-partition broadcast multiply, then DMA-transposed to [d, tok]
 - engine split: PE matmuls; DVE does every PSUM read (center/mu/acc/bias -
   GPSIMD cannot access PSUM) plus the Schraudolph exps; Pool does SBUF-only
   elementwise; ACT does exp plus the prologue squares (Square shares the
   exp table set, no reload)
"""

import sys

for _p in ("/opt/trn_rl_repo",):
    if _p not in sys.path:
        sys.path.insert(0, _p)

import numpy as np
import ml_dtypes
from contextlib import ExitStack

import concourse.bass as bass
import concourse.bacc as bacc
import concourse.tile as tile
from concourse import mybir
from concourse import bass_utils

BF16 = ml_dtypes.bfloat16

B, S, DM = 4, 2048, 1024
H, HD = 16, 64
NCORES = 8
SQ = S // 2          # query tokens per core
NT_K = S // 128      # 16 token tiles for k/v
NT_Q = SQ // 128     # 8 token tiles for q
NIT = DM // 128      # 8 contraction tiles
NOC = DM // 512      # 2 output column chunks
QB = 512             # query block width in attention
NQB = SQ // QB       # 2
LN_EPS = 1e-5
SCW = 256.0
SCX = 8.0
SC2 = (SCW * SCX) ** 2

# which projections keep the Wlo fp8 half
W_LO = {"q": True, "k": True, "v": True}
# qb1 kt's whose exp runs on DVE (Schraudolph) instead of ACT
SCHR_KTS = (2, 4, 7, 9, 12, 14)
# bf16-bits Schraudolph: bits(exp(u)) ~ 184.665*u + 16256 - C; u = s/8;
# +0.5 folds the int16 truncation into round-to-nearest
SCHR_A = 184.66496 / 8.0
SCHR_B = 16256.0 - 0.043 * 128 + 0.5


def _build_program():
    nc = bacc.Bacc("TRN2", target_bir_lowering=False, debug=False)

    f32 = mybir.dt.float32
    bf16 = mybir.dt.bfloat16
    i16 = mybir.dt.int16
    f8 = mybir.dt.float8e4

    xh_d = nc.dram_tensor("xh", [DM, S], f8, kind="ExternalInput").ap()
    xl_d = nc.dram_tensor("xl", [DM, S], f8, kind="ExternalInput").ap()
    w_d = {}
    for n in ("q", "k", "v"):
        for half in ("h",) + (("l",) if W_LO[n] else ()):
            w_d[n + half] = nc.dram_tensor(
                f"w{n}{half}", [DM, DM + H], f8, kind="ExternalInput"
            ).ap()
    w_d["o"] = nc.dram_tensor("wo", [DM, DM], bf16, kind="ExternalInput").ap()
    bs_d = nc.dram_tensor("bs", [3, DM + H], bf16, kind="ExternalInput").ap()
    bo_d = nc.dram_tensor("bo", [1, DM], bf16, kind="ExternalInput").ap()
    out_d = nc.dram_tensor("out", [SQ, DM], f32, kind="ExternalOutput").ap()

    with ExitStack() as ctx:
        tc = ctx.enter_context(tile.TileContext(nc))

        consts = ctx.enter_context(tc.tile_pool(name="consts", bufs=2))
        qT_p = ctx.enter_context(tc.tile_pool(name="qT", bufs=1))
        kT_p = ctx.enter_context(tc.tile_pool(name="kT", bufs=1))
        vA_p = ctx.enter_context(tc.tile_pool(name="vA", bufs=1))
        stage_p = ctx.enter_context(tc.tile_pool(name="stage", bufs=2))
        stagebf_p = ctx.enter_context(tc.tile_pool(name="stagebf", bufs=1))
        sq_p = ctx.enter_context(tc.tile_pool(name="sq", bufs=1))
        stats_p = ctx.enter_context(tc.tile_pool(name="stats", bufs=4))
        probs_p = ctx.enter_context(tc.tile_pool(name="probs", bufs=3))
        a2_p = ctx.enter_context(tc.tile_pool(name="a2", bufs=4))

        qT = qT_p.tile([128, NIT, SQ], bf16)    # [d-part, head-pair, q-token]
        kT = kT_p.tile([128, NIT, S], bf16)     # [d-part, head-pair, k-token]
        vA = vA_p.tile([128, NT_K, H, HD + 1], bf16)  # [k-part, kt, head, d+1]

        nc.vector.memset(vA[:, :, :, HD:HD + 1], 1.0)

        i32 = mybir.dt.int32
        magic_t = consts.tile([128, H], i32, tag="magic", bufs=1)
        nc.vector.memset(magic_t, 0x5f3759df)

        def bcast3(t):
            return bass.AP(
                tensor=t.tensor, offset=t.offset,
                ap=[t.ap[0], t.ap[1], [0, HD]],
            )

        def load_w(name, pool, w_pool=None, bias_bufs=3):
            if name == "o":
                wt = pool.tile([128, NIT, DM], bf16, tag="wo")
                nc.sync.dma_start(
                    out=wt,
                    in_=w_d["o"].rearrange("(t p) o -> p t o", p=128),
                )
                return wt, None
            wh = w_pool.tile([128, NIT, DM + H], f8, tag="wh", bufs=3,
                             name=f"wh_{name}")
            nc.sync.dma_start(
                out=wh,
                in_=w_d[name + "h"].rearrange("(t p) o -> p t o", p=128),
            )
            if W_LO[name]:
                wl = w_pool.tile([128, NIT, DM + H], f8, tag="wl", bufs=3,
                                 name=f"wl_{name}")
                nc.sync.dma_start(
                    out=wl,
                    in_=w_d[name + "l"].rearrange("(t p) o -> p t o", p=128),
                )
            else:
                wl = None
            bt = consts.tile([128, DM + H], bf16, tag="bias", bufs=bias_bufs,
                             name=f"b{name}")
            row = {"q": 0, "k": 1, "v": 2}[name]
            src = bs_d[row:row + 1, :]
            nc.gpsimd.dma_start(
                out=bt,
                in_=bass.AP(tensor=src.tensor, offset=src.offset,
                            ap=[[0, 128], src.ap[-1]]),
            )
            return (wh, wl), bt

        DR = mybir.MatmulPerfMode.DoubleRow

        def make_proj(xTh, xTl):
            def dr_chain(out, csl, tsl, whl):
                """y[tsl, csl] = W@x via fp8 DoubleRow matmuls. hi-parts
                first so the xl-dependent tail tolerates a late xl DMA."""
                wh, wl = whl
                parts = [(xTh, wh)]
                if wl is not None:
                    parts.append((xTh, wl))
                parts.append((xTl, wh))
                seq = [(m, s, v) for (s, v) in parts for m in range(NIT // 2)]
                for idx, (m, stat, mov) in enumerate(seq):
                    ksl = slice(2 * m, 2 * m + 2)
                    nc.tensor.matmul(
                        out, stat[:, ksl, tsl], mov[:, ksl, csl],
                        start=(idx == 0), stop=(idx == len(seq) - 1),
                        perf_mode=DR,
                    )

            def proj_pieces(name, whl, bt, tt, pool, square_act=False,
                            sq_pool=None, nb_pool=None, drain_act=False):
                """One token tile of projection + LN as two ~1.3us PE pieces.

                DVE drains+centers (only DVE/ACT/PE touch PSUM), Pool does
                the SBUF-only elementwise work.
                """
                tsl = slice(tt * 128, (tt + 1) * 128)
                st = {}

                def piece_a():
                    pm = pool.tile([128, H], f32, tag="ps")
                    dr_chain(pm, slice(DM, DM + H), tsl, whl)
                    mu = stats_p.tile([128, H], f32, tag="mu")
                    nc.vector.tensor_add(out=mu, in0=pm, in1=bt[:, DM:DM + H])
                    cen = stage_p.tile([128, DM], bf16, tag="cen")
                    st["mu"], st["cen"] = mu, cen
                    ps = pool.tile([128, 512], f32, tag="ps")
                    dr_chain(ps, slice(0, 512), tsl, whl)
                    cen3a = cen.rearrange("p (h d) -> p h d", h=H)
                    if drain_act:
                        # ACT (half-idle in the frontier) drains the PSUM;
                        # Pool centers. Keeps DVE off the psA recycle path.
                        nc.scalar.copy(out=cen[:, 0:512], in_=ps)
                        nc.gpsimd.tensor_sub(
                            out=cen3a[:, 0:8, :], in0=cen3a[:, 0:8, :],
                            in1=bcast3(st["mu"])[:, 0:8, :],
                        )
                    else:
                        nc.vector.tensor_sub(
                            out=cen3a[:, 0:8, :],
                            in0=ps.rearrange("p (h d) -> p h d", h=8),
                            in1=bcast3(st["mu"])[:, 0:8, :],
                        )

                def piece_b():
                    mu, cen = st["mu"], st["cen"]
                    cen3 = cen.rearrange("p (h d) -> p h d", h=H)
                    ps = pool.tile([128, 512], f32, tag="ps")
                    dr_chain(ps, slice(512, 1024), tsl, whl)
                    if drain_act:
                        nc.scalar.copy(out=cen[:, 512:1024], in_=ps)
                        nc.gpsimd.tensor_sub(
                            out=cen3[:, 8:16, :], in0=cen3[:, 8:16, :],
                            in1=bcast3(mu)[:, 8:16, :],
                        )
                    else:
                        nc.vector.tensor_sub(
                            out=cen3[:, 8:16, :],
                            in0=ps.rearrange("p (h d) -> p h d", h=8),
                            in1=bcast3(mu)[:, 8:16, :],
                        )
                    nc.gpsimd.tensor_add(out=cen, in0=cen, in1=bt[:, 0:DM])

                    sqt = (sq_pool or sq_p).tile(
                        [128, DM], bf16, tag="probs" if sq_pool else "sq",
                        name="sqt")
                    if square_act:
                        nc.scalar.activation(
                            out=sqt, in_=cen,
                            func=mybir.ActivationFunctionType.Square,
                        )
                    else:
                        nc.gpsimd.tensor_mul(out=sqt, in0=cen, in1=cen)
                    ssq = stats_p.tile([128, H], f32, tag="ssq")
                    nc.vector.tensor_reduce(
                        out=ssq, in_=sqt.rearrange("p (h d) -> p h d", h=H),
                        axis=mybir.AxisListType.X, op=mybir.AluOpType.add,
                    )
                    var = stats_p.tile([128, H], f32, tag="var")
                    nc.vector.tensor_scalar(
                        out=var, in0=ssq, scalar1=1.0 / HD,
                        scalar2=LN_EPS * SC2,
                        op0=mybir.AluOpType.mult, op1=mybir.AluOpType.add,
                    )
                    shi = stats_p.tile([128, H], i32, tag="shi")
                    nc.vector.tensor_scalar(
                        out=shi, in0=var.bitcast(i32), scalar1=1, scalar2=None,
                        op0=mybir.AluOpType.logical_shift_right,
                    )
                    rstd = stats_p.tile([128, H], f32, tag="rstd")
                    nc.gpsimd.tensor_sub(
                        out=rstd.bitcast(i32), in0=magic_t, in1=shi)
                    nt = stats_p.tile([128, H], f32, tag="nt")
                    nc.gpsimd.tensor_mul(out=nt, in0=rstd, in1=rstd)
                    nc.gpsimd.tensor_mul(out=nt, in0=nt, in1=var)
                    nc.vector.tensor_scalar(
                        out=nt, in0=nt, scalar1=-0.5, scalar2=1.5,
                        op0=mybir.AluOpType.mult, op1=mybir.AluOpType.add,
                    )
                    nc.gpsimd.tensor_mul(out=rstd, in0=rstd, in1=nt)

                    if name == "v":
                        nc.gpsimd.tensor_mul(
                            out=vA[:, tt, :, 0:HD], in0=cen3,
                            in1=bcast3(rstd),
                        )
                    else:
                        nb = (nb_pool or stagebf_p).tile(
                            [128, DM], bf16, tag="a2" if nb_pool else "nbf",
                            name="nb")
                        nc.gpsimd.tensor_mul(
                            out=nb.rearrange("p (h d) -> p h d", h=H),
                            in0=cen3, in1=bcast3(rstd),
                        )
                        dst = qT if name == "q" else kT
                        nc.sync.dma_start_transpose(dst[:, :, tsl], nb)

                return piece_a, piece_b
            return proj_pieces

        NTT = QB // 128

        psS_ref = [None]
        psO_ref = [None]

        def scores_exp(qb, j, kt, on_dve=False, pool=None):
            pool = pool if pool is not None else probs_p
            ksl = slice(kt * 128, (kt + 1) * 128)
            qsl = slice(qb * QB, (qb + 1) * QB)
            sp = psS_ref[0].tile([128, 2, QB], f32, tag="psS")
            for hh in range(2):
                psl = slice(hh * HD, (hh + 1) * HD)
                nc.tensor.matmul(
                    sp[:, hh, :], kT[psl, j, ksl], qT[psl, j, qsl],
                    start=True, stop=True,
                )
            if on_dve:
                pti = pool.tile([128, 2, QB], i16, tag="probs")
                nc.vector.tensor_scalar(
                    out=pti, in0=sp, scalar1=SCHR_A, scalar2=SCHR_B,
                    op0=mybir.AluOpType.mult, op1=mybir.AluOpType.add,
                )
                pt = pti.bitcast(bf16)
            else:
                pt = pool.tile([128, 2, QB], bf16, tag="probs")
                nc.scalar.activation(
                    out=pt, in_=sp,
                    func=mybir.ActivationFunctionType.Exp,
                    scale=1.0 / np.sqrt(HD),
                )
            return pt

        def pv_block(oX, pt, j, kt, start, stop):
            for tt in range(NTT):
                for hh in range(2):
                    idx = (tt % 2) * 2 + hh
                    nc.tensor.matmul(
                        oX[tt // 2][:, idx, :],
                        pt[:, hh, tt * 128:(tt + 1) * 128],
                        vA[:, kt, 2 * j + hh, :],
                        start=(start and idx == 0),
                        stop=(stop and idx == 3),
                        skip_group_check=True,
                    )

        def divide_out(src_halves, a2t, j, eng):
            for half in range(2):
                src = src_halves[half]
                rden = stats_p.tile([128, 4], f32, tag="rden")
                nc.vector.reciprocal(out=rden, in_=src[:, :, HD])
                for i in range(4):
                    tt, hh = half * 2 + i // 2, i % 2
                    rsl = rden[:, i:i + 1]
                    eng.tensor_mul(
                        out=a2t[tt].rearrange(
                            "p (j hh d) -> p j hh d", j=NIT, hh=2
                        )[:, j, hh, :],
                        in0=src[:, i, 0:HD],
                        in1=bass.AP(tensor=rsl.tensor, offset=rsl.offset,
                                    ap=[rsl.ap[0], [0, HD]]),
                    )

        # ---- emission schedule ----
        # acc opens before xT/w so those two can close mid-frontier (LIFO)
        acc_p_cm = tc.tile_pool(name="acc", bufs=1)
        acc_pool = acc_p_cm.__enter__()
        ctx.push(acc_p_cm)  # closes at program end, after aT/wo/outst
        with tc.tile_pool(name="xT", bufs=1) as xT_p, \
             tc.tile_pool(name="w", bufs=1) as w_p:
            xTh = xT_p.tile([128, NIT, S], f8, tag="xh")
            xTl = xT_p.tile([128, NIT, S], f8, tag="xl")
            # DMA priority (shared DMA device serializes transfers): token
            # halves of x split so the prologue's inputs (tokens 0-1023)
            # arrive before the k/v weights, and the second halves (only
            # needed by k/v tiles 8-15, mid-frontier) come last
            xh_r = xh_d.rearrange("(t p) s -> p t s", p=128)
            xl_r = xl_d.rearrange("(t p) s -> p t s", p=128)
            nc.scalar.dma_start(out=xTh[:, :, 0:SQ], in_=xh_r[:, :, 0:SQ])
            wt_q, bt_q = load_w("q", None, w_p)
            nc.scalar.dma_start(out=xTl[:, :, 0:SQ], in_=xl_r[:, :, 0:SQ])
            wt_k, bt_k = load_w("k", None, w_p)
            proj_pieces = make_proj(xTh, xTl)

            with tc.tile_pool(name="psB", bufs=8, space="PSUM") as psB:
                for name, tt in [("q", 0), ("q", 1), ("q", 2), ("q", 3),
                                 ("k", 0)]:
                    wt, bt = {"q": (wt_q, bt_q), "k": (wt_k, bt_k)}[name]
                    a, b_ = proj_pieces(name, wt, bt, tt, psB,
                                        square_act=(name == "q"),
                                        sq_pool=probs_p, nb_pool=a2_p)
                    a()
                    b_()
                # v loads last: k0's transpose gates the whole exp stream,
                # while v0's vA only gates the first pv (the exp pipeline
                # covers v's LN latency)
                wt_v, bt_v = load_w("v", None, w_p)
                for tt in (0, 1):
                    a, b_ = proj_pieces("v", wt_v, bt_v, tt, psB,
                                        sq_pool=probs_p, nb_pool=a2_p)
                    a()
                    b_()
                # x second halves (tokens 1024-2047) are only needed by k/v
                # tiles 8-15 mid-frontier: queue them dead last via SWDGE so
                # they cannot cut ahead of wv or the startup transposes on
                # the serial DMA device
                nc.gpsimd.dma_start(out=xTh[:, :, SQ:S], in_=xh_r[:, :, SQ:S])
                nc.gpsimd.dma_start(out=xTl[:, :, SQ:S], in_=xl_r[:, :, SQ:S])

            psA = ctx.enter_context(
                tc.tile_pool(name="psA", bufs=2, space="PSUM"))
            psS_ref[0] = ctx.enter_context(
                tc.tile_pool(name="psS", bufs=2, space="PSUM"))
            psO_ref[0] = ctx.enter_context(
                tc.tile_pool(name="psO", bufs=2, space="PSUM"))
            psO = psO_ref[0]

            # frontier feed queue: the remaining projection tiles as ~1.3us
            # pieces, ordered so each step's k/v tiles land just before the
            # step that needs them; q4-7 (only needed by qb1) go last
            feed_tiles = [("k", 1), ("v", 2), ("v", 3), ("k", 2), ("k", 3),
                          ("k", 4), ("k", 5), ("v", 4), ("k", 6), ("k", 7),
                          ("v", 5), ("v", 6), ("v", 7), ("k", 8), ("k", 9),
                          ("v", 8), ("v", 9), ("k", 10), ("k", 11),
                          ("v", 10), ("v", 11), ("k", 12), ("k", 13),
                          ("v", 12), ("v", 13), ("k", 14), ("k", 15),
                          ("v", 14), ("v", 15),
                          ("q", 4), ("q", 5), ("q", 6), ("q", 7)]
            feedq = []
            for name, tt in feed_tiles:
                wt, bt = {"q": (wt_q, bt_q), "k": (wt_k, bt_k),
                          "v": (wt_v, bt_v)}[name]
                a, b_ = proj_pieces(name, wt, bt, tt, psA)
                feedq += [a, b_]

            iters = [(0,), (1,), (2, 3), (4, 5, 6, 7),
                     (8, 9, 10, 11), (12, 13, 14, 15)]

            acc = [
                acc_pool.tile([128, 2, 4, HD + 1], f32, name=f"acc{j}")
                for j in range(NIT)
            ]
            a2t0 = [
                a2_p.tile([128, DM], bf16, tag="a2", name=f"a2_0_{tt}")
                for tt in range(NTT)
            ]
            st_aT = {}

            def run_iter(it, kts, last):
                for j in range(NIT):
                    oX = [
                        psO.tile([128, 4, HD + 1], f32, tag="psO",
                                 name=f"oF{it}_{j}_{h}")
                        for h in range(2)
                    ]
                    for i, kt in enumerate(kts):
                        pt = scores_exp(0, j, kt)
                        # the feed piece runs on the PE while ACT does
                        # this sub-unit's exp; pv then has its probs
                        if feedq:
                            feedq.pop(0)()
                        pv_block(oX, pt, j, kt,
                                 start=(i == 0), stop=(i == len(kts) - 1))
                    for half in range(2):
                        if it == 0:
                            nc.vector.tensor_copy(
                                out=acc[j][:, half], in_=oX[half])
                        else:
                            nc.vector.tensor_add(
                                out=acc[j][:, half], in0=acc[j][:, half],
                                in1=oX[half])
                    if last:
                        # j's qb0 accumulator is final: divide (Pool,
                        # SBUF-only) and transpose into aT while the
                        # frontier still runs
                        divide_out([acc[j][:, 0], acc[j][:, 1]], a2t0,
                                   j, nc.gpsimd)
                        for tt in range(NTT):
                            nc.sync.dma_start_transpose(
                                st_aT["aT"][:, j, tt * 128:(tt + 1) * 128],
                                a2t0[tt][:, j * 128:(j + 1) * 128])

            # iters 0-3 consume all remaining feed pieces (64 sub-units);
            # then xT and the weight pools close so wo/aT can load while
            # the frontier's second half still runs
            for it in range(4):
                run_iter(it, iters[it], False)
            while feedq:
                feedq.pop(0)()

        aT_p = ctx.enter_context(tc.tile_pool(name="aT", bufs=1))
        wo_p = ctx.enter_context(tc.tile_pool(name="woP", bufs=1))
        outst_p = ctx.enter_context(tc.tile_pool(name="outst", bufs=2))
        # post-frontier SBUF is plentiful (xT/w freed 82KB, aT/wo take 38):
        # dv stages each qb1 sweep's PSUM accumulators into SBUF so psO
        # recycles immediately and the divides run on the idle Pool engine;
        # probs2 deepens the qb1 exp pipeline beyond the frontier's 3 slots
        dv_p = ctx.enter_context(tc.tile_pool(name="dv", bufs=2))
        probs2_p = ctx.enter_context(tc.tile_pool(name="probs2", bufs=5))
        aT = aT_p.tile([128, NIT, SQ], bf16)
        st_aT["aT"] = aT
        wo, _ = load_w("o", wo_p)
        bias_o = wo_p.tile([128, DM], bf16, tag="bias_o")
        nc.gpsimd.dma_start(
            out=bias_o,
            in_=bass.AP(tensor=bo_d.tensor, offset=bo_d.offset,
                        ap=[[0, 128], bo_d.ap[1]]),
        )
        run_iter(4, iters[4], False)
        run_iter(5, iters[5], True)

        def outproj_tile(tt):
            for oc in range(NOC):
                ps = psA.tile([128, 512], f32, tag="ps")
                for j in range(NIT):
                    nc.tensor.matmul(
                        ps,
                        aT[:, j, tt * 128:(tt + 1) * 128],
                        wo[:, j, oc * 512:(oc + 1) * 512],
                        start=(j == 0), stop=(j == NIT - 1),
                    )
                ot = outst_p.tile([128, 512], f32, tag="outst")
                nc.vector.tensor_add(
                    out=ot, in0=ps,
                    in1=bias_o[:, oc * 512:(oc + 1) * 512],
                )
                nc.sync.dma_start(
                    out=out_d[tt * 128:(tt + 1) * 128,
                              oc * 512:(oc + 1) * 512],
                    in_=ot,
                )

        a2t1 = [
            a2_p.tile([128, DM], bf16, tag="a2", name=f"a2_1_{tt}")
            for tt in range(NTT)
        ]
        # qb0's out-projection as a filler stream: one matmul per qb1
        # sub-unit keeps the PE busy during each exp's latency
        fill = []
        opst = {}

        def op_mm(tt, oc, j):
            def f():
                if j == 0:
                    opst[(tt, oc)] = psA.tile([128, 512], f32, tag="ps",
                                              name=f"op{tt}_{oc}")
                ps = opst[(tt, oc)]
                nc.tensor.matmul(
                    ps, aT[:, j, tt * 128:(tt + 1) * 128],
                    wo[:, j, oc * 512:(oc + 1) * 512],
                    start=(j == 0), stop=(j == NIT - 1),
                )
                if j == NIT - 1:
                    ot = outst_p.tile([128, 512], f32, tag="outst")
                    nc.vector.tensor_add(
                        out=ot, in0=ps,
                        in1=bias_o[:, oc * 512:(oc + 1) * 512],
                    )
                    nc.sync.dma_start(
                        out=out_d[tt * 128:(tt + 1) * 128,
                                  oc * 512:(oc + 1) * 512],
                        in_=ot,
                    )
            return f

        for tt in range(NTT):
            for oc in range(NOC):
                for j in range(NIT):
                    fill.append(op_mm(tt, oc, j))

        for j in range(NIT):
            oX = [
                psO.tile([128, 4, HD + 1], f32, tag="psO", name=f"oS{j}_{h}")
                for h in range(2)
            ]
            pts = [scores_exp(1, j, 0, on_dve=(0 in SCHR_KTS), pool=probs2_p),
                   scores_exp(1, j, 1, on_dve=(1 in SCHR_KTS), pool=probs2_p)]
            for kt in range(NT_K):
                if kt + 2 < NT_K:
                    pts.append(scores_exp(1, j, kt + 2,
                                          on_dve=((kt + 2) in SCHR_KTS),
                                          pool=probs2_p))
                if fill and kt % 2 == 0:
                    fill.pop(0)()
                pv_block(oX, pts[kt], j, kt,
                         start=(kt == 0), stop=(kt == NT_K - 1))
            # two quick DVE copies free the psO banks for the next sweep;
            # the divides then run on the otherwise-idle Pool engine
            dv = dv_p.tile([128, 2, 4, HD + 1], f32, tag="dv")
            nc.vector.tensor_copy(out=dv[:, 0], in_=oX[0])
            nc.vector.tensor_copy(out=dv[:, 1], in_=oX[1])
            divide_out([dv[:, 0], dv[:, 1]], a2t1, j, nc.gpsimd)
            for tt in range(NTT):
                nc.sync.dma_start_transpose(
                    aT[:, j, QB + tt * 128:QB + (tt + 1) * 128],
                    a2t1[tt][:, j * 128:(j + 1) * 128])
        for tt in range(NTT):
            outproj_tile(NTT + tt)

    nc.compile()
    return nc


_CACHE = {}


def _get_program():
    if "nc" not in _CACHE:
        _CACHE["nc"] = _build_program()
    return _CACHE["nc"]


import ml_dtypes as _mld

F8 = _mld.float8_e4m3


def _split8(a):
    hi = a.astype(F8)
    lo = (a - hi.astype(np.float32)).astype(F8)
    return np.ascontiguousarray(hi), np.ascontiguousarray(lo)


def _augment(W):
    W = np.asarray(W, dtype=np.float32)
    Wm = W.reshape(DM, H, HD).mean(axis=2)
    return np.concatenate([W, Wm], axis=1)


def _bias_row(b):
    b = np.asarray(b, dtype=np.float32)
    return np.concatenate([b, b.reshape(H, HD).mean(axis=1)])


def _make_in_maps(x, Wq, bq, Wk, bk, Wv, bv, Wo, bo):
    ws = {}
    for n, W in (("q", Wq), ("k", Wk), ("v", Wv)):
        hi, lo = _split8(_augment(W) * SCW)
        ws[f"w{n}h"] = hi
        if W_LO[n]:
            ws[f"w{n}l"] = lo
    sc = SCW * SCX
    bs = np.ascontiguousarray(
        np.stack([_bias_row(bq) * sc, _bias_row(bk) * sc,
                  _bias_row(bv) * sc]).astype(BF16))
    wo = np.ascontiguousarray(np.asarray(Wo).astype(BF16))
    bo_a = np.ascontiguousarray(
        np.asarray(bo, dtype=np.float32).reshape(1, DM).astype(BF16))
    in_maps = []
    for c in range(NCORES):
        b, hf = divmod(c, 2)
        xb = np.asarray(x[b])
        if hf:
            xb = np.concatenate([xb[SQ:], xb[:SQ]], axis=0)
        xh, xl = _split8(np.ascontiguousarray(xb.T.astype(np.float32)) * SCX)
        m = {"xh": xh, "xl": xl, "wo": wo, "bs": bs, "bo": bo_a}
        m.update(ws)
        in_maps.append(m)
    return in_maps


def _run(x, Wq, bq, Wk, bk, Wv, bv, Wo, bo, **run_kwargs):
    nc = _get_program()
    in_maps = _make_in_maps(x, Wq, bq, Wk, bk, Wv, bv, Wo, bo)
    res = bass_utils.run_bass_kernel_spmd(
        nc, in_maps, core_ids=list(range(NCORES)), **run_kwargs
    )
    out = np.empty((B, S, DM), dtype=np.float32)
    for c in range(NCORES):
        b, hf = divmod(c, 2)
        out[b, hf * SQ:(hf + 1) * SQ] = res.results[c]["out"]
    return out, res


def kernel(x, Wq, bq, Wk, bk, Wv, bv, Wo, bo):
    out, _ = _run(x, Wq, bq, Wk, bk, Wv, bv, Wo, bo)
    return out


def kernel_profiled(x, Wq, bq, Wk, bk, Wv, bv, Wo, bo):
    return _run(x, Wq, bq, Wk, bk, Wv, bv, Wo, bo, trace=True)
